# revision 8
# baseline (speedup 1.0000x reference)
"""Invariant Point Attention on 8 Trainium2 NeuronCores (Bass/Tile).

Sharding: residue i-dimension split across the 8 cores (96 rows each), params
replicated; each core attends its queries against the full key/value set.

Device program per core (all matmuls fp16 operands, fp32 PSUM accumulate):
  - softmax multiplicative split: e = exp(qk_logits) * exp(pair_bias); each
    factor is computed in its natural matmul layout:
      eq_T [j, (h,i)]   per-(h, jblk) matmuls, K=30 packed into 32-row strips
      eb   [(c,h),(i,j)] W_pair.T @ pair_T, 4-way column-tiled on the PE
  - the pair tensor is loaded twice: natural [j, d] tiles (rpair aggregation)
    and transposed [d, (i,j)] via the hardware DMA-transpose (bias matmul)
  - the q2/k2/point-distance terms are folded into the qk contraction as two
    extra K rows; normalizer from a ones-vector matmul over e_T partitions;
    1/s replicated across partitions with a K=1 outer-product matmul
  - aggregations (rs, rpt, rpair) contract j on partitions; rpair output is
    relayouted with an HBM DMA-transpose roundtrip; local-frame rotation and
    norms run on DVE/ACT batched over (strip, d) rows
  - output projection accumulates 27 permuted 128-row chunks of W_out, final
    [384, 96] -> [96, 384] via PE transpose
"""

import os
import numpy as np
from concurrent.futures import ThreadPoolExecutor

H, SKD, SVD, PKD, PVD, DIM, PD = 12, 16, 16, 4, 8, 384, 128
EPS = 1e-8
SCALAR_SCALE = (3 * SKD) ** -0.5
POINT_SCALE = (3 * PKD * (9 / 2)) ** -0.5
PAIR_SCALE = 3 ** -0.5
N_CORES = 8
N = 768
NI = N // N_CORES          # 96 residues per core
NG = NI // 4               # 24 groups of 4 residues
JB = N // 128              # 6 j-blocks
NCHUNK = 27                # feats chunks: rs 3, loc 9, norm 3, rpair 12
FDIM = NCHUNK * 128

_STATE = {}


def _build_program():
    import concourse.bacc as bacc
    import concourse.bass as bass
    import concourse.tile as tile
    from concourse import mybir

    f32, f16 = mybir.dt.float32, mybir.dt.float16
    AF = mybir.ActivationFunctionType

    def bc3(t, idx):
        # t [128, k, 96] -> slice idx broadcast x3 over a middle free dim
        ap = t[:, idx, :]
        return bass.AP(ap.tensor, ap.offset, [list(ap.ap[0]), [0, 3], [1, NI]])

    nc = bacc.Bacc("TRN2", target_bir_lowering=False, debug=False)

    PAIR = nc.dram_tensor("PAIR", [NI * N, PD], f16, kind="ExternalInput")
    K3 = nc.dram_tensor("K3", [3 * 128, N], f16, kind="ExternalInput")
    Q3 = nc.dram_tensor("Q3", [3 * 128, NI], f16, kind="ExternalInput")
    VT = nc.dram_tensor("VT", [N, 40 * H], f16, kind="ExternalInput")
    WP = nc.dram_tensor("WP", [128, H], f16, kind="ExternalInput")
    WO = nc.dram_tensor("WO", [FDIM, DIM], f16, kind="ExternalInput")
    ROT = nc.dram_tensor("ROT", [9 * 128, NI], f32, kind="ExternalInput")
    TRE = nc.dram_tensor("TRE", [3 * 128, NI], f32, kind="ExternalInput")
    EYE = nc.dram_tensor("EYE", [128, 128], f32, kind="ExternalInput")
    BOUT = nc.dram_tensor("BOUT", [DIM, 1], f32, kind="ExternalInput")
    PBIAS = nc.dram_tensor("PBIAS", [128, 1], f32, kind="ExternalInput")
    OUT = nc.dram_tensor("OUT", [NI, DIM], f32, kind="ExternalOutput")

    with tile.TileContext(nc) as tc:
        with tc.tile_pool(name="const", bufs=1) as cp, \
             tc.tile_pool(name="dram", bufs=1, space="DRAM") as dr:
            k3 = cp.tile([128, 3, N], f16)
            nc.sync.dma_start(k3[:], K3[:].rearrange("(k p) n -> p k n", p=128))
            q3 = cp.tile([128, 3, NI], f16)
            nc.sync.dma_start(q3[:], Q3[:].rearrange("(k p) n -> p k n", p=128))
            vt = cp.tile([128, JB, 40 * H], f16)
            nc.sync.dma_start(vt[:], VT[:].rearrange("(k p) n -> p k n", p=128))
            wp = cp.tile([128, H], f16)
            nc.sync.dma_start(wp[:], WP[:])
            wo = cp.tile([128, NCHUNK, DIM], f16)
            nc.sync.dma_start(wo[:], WO[:].rearrange("(k p) n -> p k n", p=128))
            rot = cp.tile([128, 9, NI], f32)
            nc.sync.dma_start(rot[:], ROT[:].rearrange("(k p) n -> p k n", p=128))
            tre = cp.tile([128, 3, NI], f32)
            nc.sync.dma_start(tre[:], TRE[:].rearrange("(k p) n -> p k n", p=128))
            eye = cp.tile([128, 128], f32)
            nc.sync.dma_start(eye[:], EYE[:])
            bout = cp.tile([128, 3, 1], f32)
            nc.sync.dma_start(bout[:], BOUT[:].rearrange("(k p) n -> p k n", p=128))
            pbias = cp.tile([128, 1], f32)
            nc.sync.dma_start(pbias[:], PBIAS[:])
            ones_col = cp.tile([128, 1], f16)
            nc.gpsimd.memset(ones_col[:], 1.0)
            ones_row = cp.tile([1, 128], f16)
            nc.gpsimd.memset(ones_row[:], 1.0)
            zb = cp.tile([128, 1], f32)
            nc.gpsimd.memset(zb[:], 0.0)
            epsb = cp.tile([128, 1], f32)
            nc.gpsimd.memset(epsb[:], EPS)

            eb_hbm = dr.tile([H * NI, N], f16)
            rpr_hbm = dr.tile([NI * H, PD], f16)

            eb_sb = cp.tile([128, NG, N], f16)
            e_t = cp.tile([128, JB, H * NI], f16)
            eb_t = cp.tile([128, JB, H * NI], f16)

            # ---- phase A: eb = exp(PAIR_SCALE * (pair @ W_pair + b_pair)) ----
            with tc.tile_pool(name="pairT", bufs=3) as ptp, \
                 tc.tile_pool(name="ebps", bufs=4, space="PSUM") as ebps:
                for g in range(NG):
                    pt = ptp.tile([128, 4 * N], f16, tag="pt")
                    nc.sync.dma_start_transpose(
                        pt[:], PAIR[g * 4 * N:(g + 1) * 4 * N, :])
                    for half in range(2):
                        pe = ebps.tile([128, 384], f32, tag="ebp")
                        for c in range(4):
                            nc.tensor.matmul(
                                pe[32 * c:32 * c + H, :], wp[:],
                                pt[:, c * N + half * 384:c * N + (half + 1) * 384],
                                start=True, stop=True, tile_position=(0, 32 * c))
                        nc.scalar.activation(
                            eb_sb[:, g, half * 384:(half + 1) * 384], pe[:],
                            AF.Exp, bias=pbias[:], scale=PAIR_SCALE)
            # eb -> HBM rows (h*96 + 4g + c), then transposed -> eb_T [j, (h,i)]
            ebv = eb_hbm[:].rearrange("(h g c) j -> c h g j", h=H, g=NG, c=4)
            for c in range(4):
                nc.sync.dma_start(ebv[c], eb_sb[32 * c:32 * c + H, :, :])
            for jb in range(JB):
                nc.sync.dma_start_transpose(
                    e_t[:, jb, :], eb_hbm[:, jb * 128:(jb + 1) * 128])
            # note: transposed eb lands in e_t; eb_t holds eq before the merge
            # (naming swap is intentional: mul below writes e_t in place)

            # ---- phase B: eq_T = exp(qk logits); e = eq*eb; normalizer ----
            with tc.tile_pool(name="eqps", bufs=1, space="PSUM") as eqps, \
                 tc.tile_pool(name="sps", bufs=1, space="PSUM") as sps:
                s_ps = [sps.tile([1, 384], f32, tag=f"s{t}", name=f"s_ps{t}")
                        for t in range(3)]
                for jb in range(JB):
                    eq = [eqps.tile([128, 384], f32, tag=f"eq{q}", name=f"eq{q}")
                          for q in range(3)]
                    for pk in range(3):
                        for s in range(4):
                            nc.tensor.matmul(
                                eq[pk][:, s * 96:(s + 1) * 96],
                                k3[32 * s:32 * s + 30, pk,
                                   jb * 128:(jb + 1) * 128],
                                q3[32 * s:32 * s + 30, pk, :],
                                start=True, stop=True,
                                tile_position=(32 * s, 0))
                    for q in range(3):
                        nc.scalar.activation(
                            eb_t[:, jb, q * 384:(q + 1) * 384], eq[q][:],
                            AF.Exp, bias=zb[:], scale=1.0)
                    nc.vector.tensor_mul(e_t[:, jb, :], e_t[:, jb, :],
                                         eb_t[:, jb, :])
                    for t in range(3):
                        nc.tensor.matmul(
                            s_ps[t][:], ones_col[:],
                            e_t[:, jb, t * 384:(t + 1) * 384],
                            start=(jb == 0), stop=(jb == JB - 1))
                s_sb = cp.tile([1, H * NI], f32)
                for t in range(3):
                    nc.vector.tensor_copy(s_sb[:, t * 384:(t + 1) * 384],
                                          s_ps[t][:])
            rcp = cp.tile([1, H * NI], f32)
            nc.vector.reciprocal(rcp[:], s_sb[:])
            rcp16 = cp.tile([1, H * NI], f16)
            nc.vector.tensor_copy(rcp16[:], rcp[:])
            sc_sb = cp.tile([128, H * NI], f16)
            with tc.tile_pool(name="scps", bufs=3, space="PSUM") as scps:
                for t in range(3):
                    sp = scps.tile([128, 384], f32, tag="sc")
                    nc.tensor.matmul(sp[:], ones_row[:],
                                     rcp16[:, t * 384:(t + 1) * 384],
                                     start=True, stop=True)
                    nc.scalar.activation(sc_sb[:, t * 384:(t + 1) * 384],
                                         sp[:], AF.Copy)
            for jb in range(JB):
                nc.vector.tensor_mul(e_t[:, jb, :], e_t[:, jb, :], sc_sb[:])

            # ---- phase C: aggregations ----
            feats = cp.tile([128, NCHUNK, NI], f16)
            nc.gpsimd.memset(feats[:], 0.0)
            vp_all = cp.tile([128, 9, NI], f32)
            nc.gpsimd.memset(vp_all[:], 0.0)

            # rs: per (h, jb) matmul into strip rows 32s..32s+16
            with tc.tile_pool(name="vsps", bufs=3, space="PSUM") as vsps:
                for q in range(3):
                    ps = vsps.tile([128, NI], f32, tag="vs")
                    for s in range(4):
                        h = 4 * q + s
                        for jb in range(JB):
                            nc.tensor.matmul(
                                ps[32 * s:32 * s + 16, :],
                                vt[:, jb, 40 * h:40 * h + 16],
                                e_t[:, jb, h * NI:(h + 1) * NI],
                                start=(jb == 0), stop=(jb == JB - 1),
                                tile_position=(0, 32 * s))
                    for s in range(4):
                        nc.vector.tensor_copy(
                            feats[32 * s:32 * s + 16, q, :],
                            ps[32 * s:32 * s + 16, :])

            # rpt: tile tq = 3c + q, strip s = h % 4 (h = 4q + s)
            with tc.tile_pool(name="vpps", bufs=3, space="PSUM") as vpps:
                for tq in range(9):
                    c, q = tq // 3, tq % 3
                    ps = vpps.tile([128, NI], f32, tag="vp")
                    for s in range(4):
                        h = 4 * q + s
                        for jb in range(JB):
                            nc.tensor.matmul(
                                ps[32 * s:32 * s + 8, :],
                                vt[:, jb,
                                   40 * h + 16 + 8 * c:40 * h + 16 + 8 * (c + 1)],
                                e_t[:, jb, h * NI:(h + 1) * NI],
                                start=(jb == 0), stop=(jb == JB - 1),
                                tile_position=(0, 32 * s))
                    for s in range(4):
                        nc.vector.tensor_copy(
                            vp_all[32 * s:32 * s + 8, tq, :],
                            ps[32 * s:32 * s + 8, :])

            # local frame: cent_c = vp_all_c - t; loc_r = sum_c cent_c * rot_cr
            cent = cp.tile([128, 9, NI], f32)
            loc = cp.tile([128, 9, NI], f32)
            tmp = cp.tile([128, 3, NI], f32)
            for c in range(3):
                nc.vector.tensor_sub(cent[:, 3 * c:3 * c + 3, :],
                                     vp_all[:, 3 * c:3 * c + 3, :], bc3(tre, c))
            for r in range(3):
                nc.vector.tensor_mul(loc[:, 3 * r:3 * r + 3, :],
                                     cent[:, 0:3, :], bc3(rot, r))
                for c in (1, 2):
                    nc.vector.tensor_mul(tmp[:], cent[:, 3 * c:3 * c + 3, :],
                                         bc3(rot, 3 * c + r))
                    nc.vector.tensor_add(loc[:, 3 * r:3 * r + 3, :],
                                         loc[:, 3 * r:3 * r + 3, :], tmp[:])
                nc.vector.tensor_copy(feats[:, 3 + 3 * r:6 + 3 * r, :],
                                      loc[:, 3 * r:3 * r + 3, :])
            sq = cp.tile([128, 3, NI], f32)
            nc.vector.tensor_mul(sq[:], loc[:, 0:3, :], loc[:, 0:3, :])
            for r in (1, 2):
                nc.vector.tensor_mul(tmp[:], loc[:, 3 * r:3 * r + 3, :],
                                     loc[:, 3 * r:3 * r + 3, :])
                nc.vector.tensor_add(sq[:], sq[:], tmp[:])
            nc.scalar.activation(feats[:, 12:15, :], sq[:], AF.Sqrt, bias=epsb[:])

            # rpair: per group of 4 residues, 4-way column-tiled
            rpair_sb = cp.tile([128, NG, PD], f16)
            e_tv = e_t[:].rearrange("p jb (h i) -> p jb h i", h=H)
            with tc.tile_pool(name="pairN", bufs=4) as pnp, \
                 tc.tile_pool(name="rpps", bufs=2, space="PSUM") as rpps:
                pv = PAIR[:].rearrange("(x p) d -> p x d", p=128)
                for g in range(NG):
                    pn = pnp.tile([128, 24, PD], f16, tag="pn")
                    nc.sync.dma_start(pn[:], pv[:, g * 24:(g + 1) * 24, :])
                    ps = rpps.tile([128, PD], f32, tag="rp")
                    for jb in range(JB):
                        for c in range(4):
                            nc.tensor.matmul(
                                ps[32 * c:32 * c + H, :],
                                e_tv[:, jb, :, g * 4 + c],
                                pn[:, c * JB + jb, :],
                                start=(jb == 0), stop=(jb == JB - 1),
                                tile_position=(0, 32 * c))
                    for c in range(4):
                        nc.vector.tensor_copy(
                            rpair_sb[32 * c:32 * c + H, g, :],
                            ps[32 * c:32 * c + H, :])
            # rpair -> HBM rows (4g + c)*12 + h -> transpose -> [d, (i,h)]
            rpv = rpr_hbm[:].rearrange("(g c h) d -> c h g d", g=NG, c=4, h=H)
            for c in range(4):
                nc.sync.dma_start(rpv[c], rpair_sb[32 * c:32 * c + H, :, :])
            rpt_t = cp.tile([128, NI * H], f16)
            nc.sync.dma_start_transpose(rpt_t[:], rpr_hbm[:])
            rptv = rpt_t[:].rearrange("p (i h) -> p i h", h=H)
            for h in range(H):
                nc.vector.tensor_copy(feats[:, 15 + h, :], rptv[:, :, h])

            # ---- output projection + final transpose ----
            out_t = cp.tile([128, 3, NI], f32)
            with tc.tile_pool(name="ops", bufs=3, space="PSUM") as ops:
                for m in range(3):
                    po = ops.tile([128, NI], f32, tag="op")
                    for k in range(NCHUNK):
                        nc.tensor.matmul(
                            po[:], wo[:, k, m * 128:(m + 1) * 128],
                            feats[:, k, :],
                            start=(k == 0), stop=(k == NCHUNK - 1))
                    nc.scalar.activation(out_t[:, m, :], po[:], AF.Identity,
                                         bias=bout[:, m, :])
            out_sb = cp.tile([NI, DIM], f32)
            with tc.tile_pool(name="tps", bufs=3, space="PSUM") as tps:
                for m in range(3):
                    pt2 = tps.tile([NI, 128], f32, tag="tp")
                    nc.tensor.transpose(pt2[:], out_t[:, m, :], eye[:])
                    nc.vector.tensor_copy(out_sb[:, m * 128:(m + 1) * 128],
                                          pt2[:])
            nc.sync.dma_start(OUT[:], out_sb[:])

    nc.compile()
    return nc


# ---------------------------------------------------------------------------
# host side
# ---------------------------------------------------------------------------

def _softplus(x):
    return np.logaddexp(0.0, x)


def _host_prep(inputs):
    f = np.float32
    x = np.asarray(inputs["single_repr"], f)[0]
    rot = np.asarray(inputs["rotations"], f)[0]
    trans = np.asarray(inputs["translations"], f)[0]
    pw = _softplus(np.asarray(inputs["point_weights"], f))
    ch = POINT_SCALE * pw

    qs = (x @ np.asarray(inputs["W_sq"], f)).reshape(N, H, SKD)
    ks = (x @ np.asarray(inputs["W_sk"], f)).reshape(N, H, SKD)
    vs = (x @ np.asarray(inputs["W_sv"], f)).reshape(N, H, SVD)

    def to_global(W, d):
        p = (x @ np.asarray(W, f)).reshape(N, H, d, 3)
        return np.einsum('ihdc,irc->ihdr', p, rot) + trans[:, None, None, :]

    qp = to_global(inputs["W_pq"], PKD)
    kp = to_global(inputs["W_pk"], PKD)
    vp = to_global(inputs["W_pv"], PVD)
    q2 = np.einsum('ihdc->ih', qp * qp)
    k2 = np.einsum('ihdc->ih', kp * kp)

    # shared packs -------------------------------------------------------
    K3 = np.zeros((3, 128, N), f)
    for h in range(H):
        pk, s = divmod(h, 4)
        r0 = 32 * s
        K3[pk, r0:r0 + 16] = ks[:, h, :].T
        K3[pk, r0 + 16:r0 + 28] = kp[:, h].reshape(N, 12).T
        K3[pk, r0 + 28] = -0.5 * ch[h] * k2[:, h]
        K3[pk, r0 + 29] = 1.0
    K3 = K3.reshape(384, N).astype(np.float16)

    VT = np.zeros((N, H, 40), f)
    VT[:, :, :16] = vs
    VT[:, :, 16:40] = vp.transpose(0, 1, 3, 2).reshape(N, H, 24)  # (c, d)
    VT = VT.reshape(N, 40 * H).astype(np.float16)

    WP = np.asarray(inputs["W_pair"], f).astype(np.float16)        # [128, 12]

    W_out = np.asarray(inputs["W_out"], f)
    WO = np.zeros((NCHUNK, 128, DIM), f)
    for h in range(H):
        q, s = divmod(h, 4)
        WO[q, 32 * s:32 * s + 16] = W_out[16 * h:16 * h + 16]
        for r in range(3):
            for d in range(PVD):
                WO[3 + 3 * r + q, 32 * s + d] = W_out[192 + 24 * h + 3 * d + r]
        WO[12 + q, 32 * s:32 * s + 8] = W_out[480 + 8 * h:480 + 8 * h + 8]
        WO[15 + h, :] = W_out[576 + 128 * h:576 + 128 * (h + 1)]
    WO = WO.reshape(FDIM, DIM).astype(np.float16)

    PBIAS = np.zeros((128, 1), f)
    b_pair = np.asarray(inputs["b_pair"], f)
    for c in range(4):
        PBIAS[32 * c:32 * c + H, 0] = PAIR_SCALE * b_pair
    BOUT = np.asarray(inputs["b_out"], f).reshape(DIM, 1)
    EYE = np.eye(128, dtype=f)

    # per-core packs -----------------------------------------------------
    pair_full = np.asarray(inputs["pairwise_repr"])[0]
    qsT = qs.transpose(1, 2, 0)
    qpT = qp.reshape(N, H, 12).transpose(1, 2, 0)

    def per_core(cc):
        i0 = cc * NI
        sl = slice(i0, i0 + NI)
        Q3 = np.zeros((3, 128, NI), f)
        for h in range(H):
            pk, s = divmod(h, 4)
            r0 = 32 * s
            Q3[pk, r0:r0 + 16] = SCALAR_SCALE * qsT[h][:, sl]
            Q3[pk, r0 + 16:r0 + 28] = ch[h] * qpT[h][:, sl]
            Q3[pk, r0 + 28] = 1.0
            Q3[pk, r0 + 29] = -0.5 * ch[h] * q2[sl, h]
        ROT = np.empty((9, 128, NI), f)
        for c in range(3):
            for r in range(3):
                ROT[3 * c + r] = rot[sl, c, r][None, :]
        TRE = np.empty((3, 128, NI), f)
        for c in range(3):
            TRE[c] = trans[sl, c][None, :]
        PAIRc = np.ascontiguousarray(pair_full[sl]).reshape(NI * N, PD)
        PAIRc = PAIRc.astype(np.float16)
        return dict(
            PAIR=PAIRc, K3=K3, Q3=Q3.reshape(384, NI).astype(np.float16),
            VT=VT, WP=WP, WO=WO,
            ROT=ROT.reshape(9 * 128, NI), TRE=TRE.reshape(3 * 128, NI),
            EYE=EYE, BOUT=BOUT, PBIAS=PBIAS)

    with ThreadPoolExecutor(max_workers=8) as ex:
        in_maps = list(ex.map(per_core, range(N_CORES)))
    return in_maps


def _kernel_numpy(**inputs):
    """CPU fallback (baseline implementation)."""
    f = np.float32
    x = np.asarray(inputs["single_repr"], f)[0]
    pair = np.asarray(inputs["pairwise_repr"], f)[0]
    rot = np.asarray(inputs["rotations"], f)[0]
    trans = np.asarray(inputs["translations"], f)[0]
    W_sq, W_sk, W_sv = (np.asarray(inputs[k], f) for k in ("W_sq", "W_sk", "W_sv"))
    W_pq, W_pk, W_pv = (np.asarray(inputs[k], f) for k in ("W_pq", "W_pk", "W_pv"))
    W_pair, b_pair = np.asarray(inputs["W_pair"], f), np.asarray(inputs["b_pair"], f)
    point_weights = np.asarray(inputs["point_weights"], f)
    W_out, b_out = np.asarray(inputs["W_out"], f), np.asarray(inputs["b_out"], f)
    n = x.shape[0]
    ks = (x @ W_sk).reshape(n, H, SKD)
    vs = (x @ W_sv).reshape(n, H, SVD)

    def to_global(t, d):
        p = t.reshape(n, H, d, 3)
        return np.einsum('ihdc,irc->ihdr', p, rot) + trans[:, None, None, :]

    kp = to_global(x @ W_pk, PKD)
    vp = to_global(x @ W_pv, PVD)
    k2 = np.sum(kp * kp, axis=(-1, -2))
    pw = _softplus(point_weights)
    ksT = ks.transpose(1, 2, 0).copy()
    kpT = kp.reshape(n, H, PKD * 3).transpose(1, 2, 0).copy()
    vsH = vs.transpose(1, 0, 2).copy()
    vpH = vp.reshape(n, H, PVD * 3).transpose(1, 0, 2).copy()
    outs = []
    for c in range(N_CORES):
        i0, i1 = c * NI, (c + 1) * NI
        il = i1 - i0
        xl = x[i0:i1]
        pair_l = pair[i0:i1]
        qs = (xl @ W_sq).reshape(il, H, SKD)
        qp = np.einsum('ihdc,irc->ihdr', (xl @ W_pq).reshape(il, H, PKD, 3),
                       rot[i0:i1]) + trans[i0:i1, None, None, :]
        qsH = qs.transpose(1, 0, 2)
        qpH = qp.reshape(il, H, PKD * 3).transpose(1, 0, 2)
        logits = (qsH @ ksT) * SCALAR_SCALE
        cross = qpH @ kpT
        bias = (pair_l.reshape(il * n, PD) @ W_pair + b_pair)
        bias = bias.reshape(il, n, H).transpose(2, 0, 1)
        logits = logits + bias * PAIR_SCALE
        q2 = np.sum(qp * qp, axis=(-1, -2))
        pdist = q2.T[:, :, None] + k2.T[:, None, :] - 2.0 * cross
        logits = logits + pdist * ((-0.5 * POINT_SCALE) * pw)[:, None, None]
        m = logits.max(axis=-1, keepdims=True)
        e = np.exp(logits - m, dtype=f)
        attn = e / e.sum(axis=-1, keepdims=True)
        rs = (attn @ vsH).transpose(1, 0, 2)
        rpt = (attn @ vpH).transpose(1, 0, 2)
        rpair = np.ascontiguousarray(attn.transpose(1, 0, 2)) @ pair_l
        rpt = rpt.reshape(il, H, PVD, 3) - trans[i0:i1, None, None, :]
        rpt_local = np.einsum('ihdc,icr->ihdr', rpt, rot[i0:i1])
        rnorm = np.sqrt(np.sum(rpt_local * rpt_local, axis=-1) + EPS)
        feats = np.concatenate([
            rs.reshape(il, H * SVD), rpt_local.reshape(il, H * PVD * 3),
            rnorm.reshape(il, H * PVD), rpair.reshape(il, H * PD)], axis=-1)
        outs.append(feats @ W_out + b_out)
    return np.concatenate(outs, axis=0)[None].astype(f)


def kernel(**inputs):
    try:
        if "nc" not in _STATE:
            import jax
            cache_dir = os.path.expanduser("~/.cache/jax_ipa_kernel")
            try:
                jax.config.update("jax_compilation_cache_dir", cache_dir)
                jax.config.update("jax_persistent_cache_min_entry_size_bytes", -1)
                jax.config.update("jax_persistent_cache_min_compile_time_secs", 0)
            except Exception:
                pass
            _STATE["nc"] = _build_program()
        from concourse.bass_utils import run_bass_kernel_spmd
        in_maps = _host_prep(inputs)
        res = run_bass_kernel_spmd(_STATE["nc"], in_maps,
                                   core_ids=list(range(N_CORES)))
        out = np.concatenate([res.results[c]["OUT"] for c in range(N_CORES)],
                             axis=0)
        return out[None].astype(np.float32)
    except Exception:
        import traceback
        traceback.print_exc()
        return _kernel_numpy(**inputs)


# revision 11
# speedup vs baseline: 1.0455x; 1.0455x over previous
"""Invariant Point Attention on 8 Trainium2 NeuronCores (Bass/Tile).

Sharding: residue i-dimension split across the 8 cores (96 rows each), params
replicated; each core attends its queries against the full key/value set.

Device program per core (all matmuls fp16 operands, fp32 PSUM accumulate):
  - softmax multiplicative split: e = exp(qk_logits) * exp(pair_bias); each
    factor is computed in its natural matmul layout:
      eq_T [j, (h,i)]   per-(h, jblk) matmuls, K=30 packed into 32-row strips
      eb   [(c,h),(i,j)] W_pair.T @ pair_T, 4-way column-tiled on the PE
  - the pair tensor is loaded twice: natural [j, d] tiles (rpair aggregation)
    and transposed [d, (i,j)] via the hardware DMA-transpose (bias matmul)
  - the q2/k2/point-distance terms are folded into the qk contraction as two
    extra K rows; normalizer from a ones-vector matmul over e_T partitions;
    1/s replicated across partitions with a K=1 outer-product matmul
  - aggregations (rs, rpt, rpair) contract j on partitions; rpair output is
    relayouted with an HBM DMA-transpose roundtrip; local-frame rotation and
    norms run on DVE/ACT batched over (strip, d) rows
  - output projection accumulates 27 permuted 128-row chunks of W_out, final
    [384, 96] -> [96, 384] via PE transpose
"""

import os
import numpy as np
from concurrent.futures import ThreadPoolExecutor

H, SKD, SVD, PKD, PVD, DIM, PD = 12, 16, 16, 4, 8, 384, 128
EPS = 1e-8
SCALAR_SCALE = (3 * SKD) ** -0.5
POINT_SCALE = (3 * PKD * (9 / 2)) ** -0.5
PAIR_SCALE = 3 ** -0.5
N_CORES = 8
N = 768
NI = N // N_CORES          # 96 residues per core
NG = NI // 4               # 24 groups of 4 residues
JB = N // 128              # 6 j-blocks
NCHUNK = 27                # feats chunks: rs 3, loc 9, norm 3, rpair 12
FDIM = NCHUNK * 128

_STATE = {}


def _build_program():
    import concourse.bacc as bacc
    import concourse.bass as bass
    import concourse.tile as tile
    from concourse import mybir

    f32, f16 = mybir.dt.float32, mybir.dt.float16
    AF = mybir.ActivationFunctionType

    def bc3(t, idx):
        # t [128, k, 96] -> slice idx broadcast x3 over a middle free dim
        ap = t[:, idx, :]
        return bass.AP(ap.tensor, ap.offset, [list(ap.ap[0]), [0, 3], [1, NI]])

    nc = bacc.Bacc("TRN2", target_bir_lowering=False, debug=False)

    PAIR = nc.dram_tensor("PAIR", [NI * N, PD], f16, kind="ExternalInput")
    K3 = nc.dram_tensor("K3", [3 * 128, N], f16, kind="ExternalInput")
    Q3 = nc.dram_tensor("Q3", [3 * 128, NI], f16, kind="ExternalInput")
    VT = nc.dram_tensor("VT", [N, 40 * H], f16, kind="ExternalInput")
    WP = nc.dram_tensor("WP", [128, 32], f16, kind="ExternalInput")
    WO = nc.dram_tensor("WO", [FDIM, DIM], f16, kind="ExternalInput")
    ROT = nc.dram_tensor("ROT", [9 * 128, NI], f32, kind="ExternalInput")
    TRE = nc.dram_tensor("TRE", [3 * 128, NI], f32, kind="ExternalInput")
    EYE = nc.dram_tensor("EYE", [128, 128], f32, kind="ExternalInput")
    BOUT = nc.dram_tensor("BOUT", [DIM, 1], f32, kind="ExternalInput")
    PBIAS = nc.dram_tensor("PBIAS", [128, 1], f32, kind="ExternalInput")
    OUT = nc.dram_tensor("OUT", [NI, DIM], f32, kind="ExternalOutput")

    with tile.TileContext(nc) as tc:
        with tc.tile_pool(name="const", bufs=1) as cp, \
             tc.tile_pool(name="dram", bufs=1, space="DRAM") as dr:
            k3 = cp.tile([128, 3, N], f16)
            nc.sync.dma_start(k3[:], K3[:].rearrange("(k p) n -> p k n", p=128))
            q3 = cp.tile([128, 3, NI], f16)
            nc.sync.dma_start(q3[:], Q3[:].rearrange("(k p) n -> p k n", p=128))
            vt = cp.tile([128, JB, 40 * H], f16)
            nc.sync.dma_start(vt[:], VT[:].rearrange("(k p) n -> p k n", p=128))
            wp = cp.tile([128, 32], f16)
            nc.sync.dma_start(wp[:], WP[:])
            wo = cp.tile([128, NCHUNK, DIM], f16)
            nc.sync.dma_start(wo[:], WO[:].rearrange("(k p) n -> p k n", p=128))
            rot = cp.tile([128, 9, NI], f32)
            nc.sync.dma_start(rot[:], ROT[:].rearrange("(k p) n -> p k n", p=128))
            tre = cp.tile([128, 3, NI], f32)
            nc.sync.dma_start(tre[:], TRE[:].rearrange("(k p) n -> p k n", p=128))
            eye = cp.tile([128, 128], f32)
            nc.sync.dma_start(eye[:], EYE[:])
            bout = cp.tile([128, 3, 1], f32)
            nc.sync.dma_start(bout[:], BOUT[:].rearrange("(k p) n -> p k n", p=128))
            pbias = cp.tile([128, 1], f32)
            nc.sync.dma_start(pbias[:], PBIAS[:])
            ones_col = cp.tile([128, 1], f16)
            nc.gpsimd.memset(ones_col[:], 1.0)
            ones_row = cp.tile([1, 128], f16)
            nc.gpsimd.memset(ones_row[:], 1.0)
            zb = cp.tile([128, 1], f32)
            nc.gpsimd.memset(zb[:], 0.0)
            epsb = cp.tile([128, 1], f32)
            nc.gpsimd.memset(epsb[:], EPS)

            eb_hbm = dr.tile([H * NI, N], f16)
            rpr_hbm = dr.tile([NI * H, PD], f16)

            eb_sb = cp.tile([128, NG, N], f16)
            e_t = cp.tile([128, JB, H * NI], f16)
            eb_t = cp.tile([128, JB, H * NI], f16)

            # ---- phase A: eb = exp(PAIR_SCALE * (pair @ W_pair + b_pair)) ----
            with tc.tile_pool(name="pairT", bufs=3) as ptp, \
                 tc.tile_pool(name="ebps", bufs=4, space="PSUM") as ebps:
                for g in range(NG):
                    pt = ptp.tile([128, 4 * N], f16, tag="pt")
                    nc.sync.dma_start_transpose(
                        pt[:], PAIR[g * 4 * N:(g + 1) * 4 * N, :])
                    for half in range(2):
                        pe = ebps.tile([128, 384], f32, tag="ebp")
                        for c in range(4):
                            nc.tensor.matmul(
                                pe[32 * c:32 * c + 32, :], wp[:],
                                pt[:, c * N + half * 384:c * N + (half + 1) * 384],
                                start=True, stop=True, tile_position=(0, 32 * c))
                        nc.scalar.activation(
                            eb_sb[:, g, half * 384:(half + 1) * 384], pe[:],
                            AF.Exp, bias=pbias[:], scale=PAIR_SCALE)
            # eb -> HBM rows (h*96 + 4g + c), then transposed -> eb_T [j, (h,i)]
            ebv = eb_hbm[:].rearrange("(h g c) j -> c h g j", h=H, g=NG, c=4)
            for c in range(4):
                nc.sync.dma_start(ebv[c], eb_sb[32 * c:32 * c + H, :, :])
            for jb in range(JB):
                nc.sync.dma_start_transpose(
                    e_t[:, jb, :], eb_hbm[:, jb * 128:(jb + 1) * 128])
            # note: transposed eb lands in e_t; eb_t holds eq before the merge
            # (naming swap is intentional: mul below writes e_t in place)

            # ---- phase B: eq_T = exp(qk logits); e = eq*eb; normalizer ----
            with tc.tile_pool(name="eqps", bufs=1, space="PSUM") as eqps, \
                 tc.tile_pool(name="sps", bufs=1, space="PSUM") as sps:
                s_ps = [sps.tile([1, 384], f32, tag=f"s{t}", name=f"s_ps{t}")
                        for t in range(3)]
                for jb in range(JB):
                    eq = [eqps.tile([128, 384], f32, tag=f"eq{q}", name=f"eq{q}")
                          for q in range(3)]
                    for pk in range(3):
                        for s in range(4):
                            nc.tensor.matmul(
                                eq[pk][:, s * 96:(s + 1) * 96],
                                k3[32 * s:32 * s + 30, pk,
                                   jb * 128:(jb + 1) * 128],
                                q3[32 * s:32 * s + 30, pk, :],
                                start=True, stop=True,
                                tile_position=(32 * s, 0))
                    for q in range(3):
                        nc.scalar.activation(
                            eb_t[:, jb, q * 384:(q + 1) * 384], eq[q][:],
                            AF.Exp, bias=zb[:], scale=1.0)
                    nc.vector.tensor_mul(e_t[:, jb, :], e_t[:, jb, :],
                                         eb_t[:, jb, :])
                    for t in range(3):
                        nc.tensor.matmul(
                            s_ps[t][:], ones_col[:],
                            e_t[:, jb, t * 384:(t + 1) * 384],
                            start=(jb == 0), stop=(jb == JB - 1))
                s_sb = cp.tile([1, H * NI], f32)
                for t in range(3):
                    nc.vector.tensor_copy(s_sb[:, t * 384:(t + 1) * 384],
                                          s_ps[t][:])
            rcp = cp.tile([1, H * NI], f32)
            nc.vector.reciprocal(rcp[:], s_sb[:])
            rcp16 = cp.tile([1, H * NI], f16)
            nc.vector.tensor_copy(rcp16[:], rcp[:])
            sc_sb = cp.tile([128, H * NI], f16)
            with tc.tile_pool(name="scps", bufs=3, space="PSUM") as scps:
                for t in range(3):
                    sp = scps.tile([128, 384], f32, tag="sc")
                    nc.tensor.matmul(sp[:], ones_row[:],
                                     rcp16[:, t * 384:(t + 1) * 384],
                                     start=True, stop=True)
                    nc.scalar.activation(sc_sb[:, t * 384:(t + 1) * 384],
                                         sp[:], AF.Copy)
            for jb in range(JB):
                nc.vector.tensor_mul(e_t[:, jb, :], e_t[:, jb, :], sc_sb[:])

            # ---- phase C: aggregations ----
            feats = cp.tile([128, NCHUNK, NI], f16)
            nc.gpsimd.memset(feats[:], 0.0)
            vp_all = cp.tile([128, 9, NI], f32)
            nc.gpsimd.memset(vp_all[:], 0.0)

            # rs: per (h, jb) matmul into strip rows 32s..32s+16
            with tc.tile_pool(name="vsps", bufs=1, space="PSUM") as vsps:
                for q in range(3):
                    pss = [vsps.tile([128, NI], f32, tag=f"vs{s}",
                                     name=f"vs{q}_{s}") for s in range(4)]
                    for jb in range(JB):
                        for s in range(4):
                            h = 4 * q + s
                            nc.tensor.matmul(
                                pss[s][32 * s:32 * s + 16, :],
                                vt[:, jb, 40 * h:40 * h + 16],
                                e_t[:, jb, h * NI:(h + 1) * NI],
                                start=(jb == 0), stop=(jb == JB - 1),
                                tile_position=(0, 32 * s))
                    for s in range(4):
                        nc.vector.tensor_copy(
                            feats[32 * s:32 * s + 16, q, :],
                            pss[s][32 * s:32 * s + 16, :])

            # rpt: tile tq = 3c + q, strip s = h % 4 (h = 4q + s)
            with tc.tile_pool(name="vpps", bufs=1, space="PSUM") as vpps:
                for tq in range(9):
                    c, q = tq // 3, tq % 3
                    pss = [vpps.tile([128, NI], f32, tag=f"vp{s}",
                                     name=f"vp{tq}_{s}") for s in range(4)]
                    for jb in range(JB):
                        for s in range(4):
                            h = 4 * q + s
                            nc.tensor.matmul(
                                pss[s][32 * s:32 * s + 8, :],
                                vt[:, jb,
                                   40 * h + 16 + 8 * c:40 * h + 16 + 8 * (c + 1)],
                                e_t[:, jb, h * NI:(h + 1) * NI],
                                start=(jb == 0), stop=(jb == JB - 1),
                                tile_position=(0, 32 * s))
                    for s in range(4):
                        nc.vector.tensor_copy(
                            vp_all[32 * s:32 * s + 8, tq, :],
                            pss[s][32 * s:32 * s + 8, :])

            # local frame: cent_c = vp_all_c - t; loc_r = sum_c cent_c * rot_cr
            cent = cp.tile([128, 9, NI], f32)
            loc = cp.tile([128, 9, NI], f32)
            tmp = cp.tile([128, 3, NI], f32)
            for c in range(3):
                nc.vector.tensor_sub(cent[:, 3 * c:3 * c + 3, :],
                                     vp_all[:, 3 * c:3 * c + 3, :], bc3(tre, c))
            for r in range(3):
                nc.vector.tensor_mul(loc[:, 3 * r:3 * r + 3, :],
                                     cent[:, 0:3, :], bc3(rot, r))
                for c in (1, 2):
                    nc.vector.tensor_mul(tmp[:], cent[:, 3 * c:3 * c + 3, :],
                                         bc3(rot, 3 * c + r))
                    nc.vector.tensor_add(loc[:, 3 * r:3 * r + 3, :],
                                         loc[:, 3 * r:3 * r + 3, :], tmp[:])
                nc.vector.tensor_copy(feats[:, 3 + 3 * r:6 + 3 * r, :],
                                      loc[:, 3 * r:3 * r + 3, :])
            sq = cp.tile([128, 3, NI], f32)
            nc.vector.tensor_mul(sq[:], loc[:, 0:3, :], loc[:, 0:3, :])
            for r in (1, 2):
                nc.vector.tensor_mul(tmp[:], loc[:, 3 * r:3 * r + 3, :],
                                     loc[:, 3 * r:3 * r + 3, :])
                nc.vector.tensor_add(sq[:], sq[:], tmp[:])
            nc.scalar.activation(feats[:, 12:15, :], sq[:], AF.Sqrt, bias=epsb[:])

            # rpair: per group of 4 residues, 4-way column-tiled
            rpair_sb = cp.tile([128, NG, PD], f16)
            e_tv = e_t[:].rearrange("p jb (h i) -> p jb h i", h=H)
            with tc.tile_pool(name="pairN", bufs=4) as pnp, \
                 tc.tile_pool(name="rpps", bufs=1, space="PSUM") as rpps:
                pv = PAIR[:].rearrange("(x p) d -> p x d", p=128)
                for g in range(NG):
                    pn = pnp.tile([128, 24, PD], f16, tag="pn")
                    nc.sync.dma_start(pn[:], pv[:, g * 24:(g + 1) * 24, :])
                    pss = [rpps.tile([128, PD], f32, tag=f"rp{c}",
                                     name=f"rp{g}_{c}") for c in range(4)]
                    for jb in range(JB):
                        for c in range(4):
                            nc.tensor.matmul(
                                pss[c][32 * c:32 * c + H, :],
                                e_tv[:, jb, :, g * 4 + c],
                                pn[:, c * JB + jb, :],
                                start=(jb == 0), stop=(jb == JB - 1),
                                tile_position=(0, 32 * c))
                    for c in range(4):
                        nc.vector.tensor_copy(
                            rpair_sb[32 * c:32 * c + H, g, :],
                            pss[c][32 * c:32 * c + H, :])
            # rpair -> HBM rows (4g + c)*12 + h -> transpose -> [d, (i,h)]
            rpv = rpr_hbm[:].rearrange("(g c h) d -> c h g d", g=NG, c=4, h=H)
            for c in range(4):
                nc.sync.dma_start(rpv[c], rpair_sb[32 * c:32 * c + H, :, :])
            rpt_t = cp.tile([128, NI * H], f16)
            nc.sync.dma_start_transpose(rpt_t[:], rpr_hbm[:])
            rptv = rpt_t[:].rearrange("p (i h) -> p i h", h=H)
            for h in range(H):
                nc.vector.tensor_copy(feats[:, 15 + h, :], rptv[:, :, h])

            # ---- output projection + final transpose ----
            out_t = cp.tile([128, 3, NI], f32)
            with tc.tile_pool(name="ops", bufs=3, space="PSUM") as ops:
                for m in range(3):
                    po = ops.tile([128, NI], f32, tag="op")
                    for k in range(NCHUNK):
                        nc.tensor.matmul(
                            po[:], wo[:, k, m * 128:(m + 1) * 128],
                            feats[:, k, :],
                            start=(k == 0), stop=(k == NCHUNK - 1))
                    nc.scalar.activation(out_t[:, m, :], po[:], AF.Identity,
                                         bias=bout[:, m, :])
            out_sb = cp.tile([NI, DIM], f32)
            with tc.tile_pool(name="tps", bufs=3, space="PSUM") as tps:
                for m in range(3):
                    pt2 = tps.tile([NI, 128], f32, tag="tp")
                    nc.tensor.transpose(pt2[:], out_t[:, m, :], eye[:])
                    nc.vector.tensor_copy(out_sb[:, m * 128:(m + 1) * 128],
                                          pt2[:])
            nc.sync.dma_start(OUT[:], out_sb[:])

    nc.compile()
    return nc


# ---------------------------------------------------------------------------
# host side
# ---------------------------------------------------------------------------

def _softplus(x):
    return np.logaddexp(0.0, x)


def _host_prep(inputs):
    f = np.float32
    x = np.asarray(inputs["single_repr"], f)[0]
    rot = np.asarray(inputs["rotations"], f)[0]
    trans = np.asarray(inputs["translations"], f)[0]
    pw = _softplus(np.asarray(inputs["point_weights"], f))
    ch = POINT_SCALE * pw

    qs = (x @ np.asarray(inputs["W_sq"], f)).reshape(N, H, SKD)
    ks = (x @ np.asarray(inputs["W_sk"], f)).reshape(N, H, SKD)
    vs = (x @ np.asarray(inputs["W_sv"], f)).reshape(N, H, SVD)

    def to_global(W, d):
        p = (x @ np.asarray(W, f)).reshape(N, H, d, 3)
        return np.einsum('ihdc,irc->ihdr', p, rot) + trans[:, None, None, :]

    qp = to_global(inputs["W_pq"], PKD)
    kp = to_global(inputs["W_pk"], PKD)
    vp = to_global(inputs["W_pv"], PVD)
    q2 = np.einsum('ihdc->ih', qp * qp)
    k2 = np.einsum('ihdc->ih', kp * kp)

    # shared packs -------------------------------------------------------
    K3 = np.zeros((3, 128, N), f)
    for h in range(H):
        pk, s = divmod(h, 4)
        r0 = 32 * s
        K3[pk, r0:r0 + 16] = ks[:, h, :].T
        K3[pk, r0 + 16:r0 + 28] = kp[:, h].reshape(N, 12).T
        K3[pk, r0 + 28] = -0.5 * ch[h] * k2[:, h]
        K3[pk, r0 + 29] = 1.0
    K3 = K3.reshape(384, N).astype(np.float16)

    VT = np.zeros((N, H, 40), f)
    VT[:, :, :16] = vs
    VT[:, :, 16:40] = vp.transpose(0, 1, 3, 2).reshape(N, H, 24)  # (c, d)
    VT = VT.reshape(N, 40 * H).astype(np.float16)

    WP = np.zeros((128, 32), np.float16)
    WP[:, :H] = np.asarray(inputs["W_pair"], f).astype(np.float16)

    W_out = np.asarray(inputs["W_out"], f)
    WO = np.zeros((NCHUNK, 128, DIM), f)
    for h in range(H):
        q, s = divmod(h, 4)
        WO[q, 32 * s:32 * s + 16] = W_out[16 * h:16 * h + 16]
        for r in range(3):
            for d in range(PVD):
                WO[3 + 3 * r + q, 32 * s + d] = W_out[192 + 24 * h + 3 * d + r]
        WO[12 + q, 32 * s:32 * s + 8] = W_out[480 + 8 * h:480 + 8 * h + 8]
        WO[15 + h, :] = W_out[576 + 128 * h:576 + 128 * (h + 1)]
    WO = WO.reshape(FDIM, DIM).astype(np.float16)

    PBIAS = np.zeros((128, 1), f)
    b_pair = np.asarray(inputs["b_pair"], f)
    for c in range(4):
        PBIAS[32 * c:32 * c + H, 0] = PAIR_SCALE * b_pair
    BOUT = np.asarray(inputs["b_out"], f).reshape(DIM, 1)
    EYE = np.eye(128, dtype=f)

    # per-core packs -----------------------------------------------------
    pair_full = np.asarray(inputs["pairwise_repr"])[0]
    qsT = qs.transpose(1, 2, 0)
    qpT = qp.reshape(N, H, 12).transpose(1, 2, 0)

    def per_core(cc):
        i0 = cc * NI
        sl = slice(i0, i0 + NI)
        Q3 = np.zeros((3, 128, NI), f)
        for h in range(H):
            pk, s = divmod(h, 4)
            r0 = 32 * s
            Q3[pk, r0:r0 + 16] = SCALAR_SCALE * qsT[h][:, sl]
            Q3[pk, r0 + 16:r0 + 28] = ch[h] * qpT[h][:, sl]
            Q3[pk, r0 + 28] = 1.0
            Q3[pk, r0 + 29] = -0.5 * ch[h] * q2[sl, h]
        ROT = np.empty((9, 128, NI), f)
        for c in range(3):
            for r in range(3):
                ROT[3 * c + r] = rot[sl, c, r][None, :]
        TRE = np.empty((3, 128, NI), f)
        for c in range(3):
            TRE[c] = trans[sl, c][None, :]
        PAIRc = np.ascontiguousarray(pair_full[sl]).reshape(NI * N, PD)
        PAIRc = PAIRc.astype(np.float16)
        return dict(
            PAIR=PAIRc, K3=K3, Q3=Q3.reshape(384, NI).astype(np.float16),
            VT=VT, WP=WP, WO=WO,
            ROT=ROT.reshape(9 * 128, NI), TRE=TRE.reshape(3 * 128, NI),
            EYE=EYE, BOUT=BOUT, PBIAS=PBIAS)

    with ThreadPoolExecutor(max_workers=8) as ex:
        in_maps = list(ex.map(per_core, range(N_CORES)))
    return in_maps


def _kernel_numpy(**inputs):
    """CPU fallback (baseline implementation)."""
    f = np.float32
    x = np.asarray(inputs["single_repr"], f)[0]
    pair = np.asarray(inputs["pairwise_repr"], f)[0]
    rot = np.asarray(inputs["rotations"], f)[0]
    trans = np.asarray(inputs["translations"], f)[0]
    W_sq, W_sk, W_sv = (np.asarray(inputs[k], f) for k in ("W_sq", "W_sk", "W_sv"))
    W_pq, W_pk, W_pv = (np.asarray(inputs[k], f) for k in ("W_pq", "W_pk", "W_pv"))
    W_pair, b_pair = np.asarray(inputs["W_pair"], f), np.asarray(inputs["b_pair"], f)
    point_weights = np.asarray(inputs["point_weights"], f)
    W_out, b_out = np.asarray(inputs["W_out"], f), np.asarray(inputs["b_out"], f)
    n = x.shape[0]
    ks = (x @ W_sk).reshape(n, H, SKD)
    vs = (x @ W_sv).reshape(n, H, SVD)

    def to_global(t, d):
        p = t.reshape(n, H, d, 3)
        return np.einsum('ihdc,irc->ihdr', p, rot) + trans[:, None, None, :]

    kp = to_global(x @ W_pk, PKD)
    vp = to_global(x @ W_pv, PVD)
    k2 = np.sum(kp * kp, axis=(-1, -2))
    pw = _softplus(point_weights)
    ksT = ks.transpose(1, 2, 0).copy()
    kpT = kp.reshape(n, H, PKD * 3).transpose(1, 2, 0).copy()
    vsH = vs.transpose(1, 0, 2).copy()
    vpH = vp.reshape(n, H, PVD * 3).transpose(1, 0, 2).copy()
    outs = []
    for c in range(N_CORES):
        i0, i1 = c * NI, (c + 1) * NI
        il = i1 - i0
        xl = x[i0:i1]
        pair_l = pair[i0:i1]
        qs = (xl @ W_sq).reshape(il, H, SKD)
        qp = np.einsum('ihdc,irc->ihdr', (xl @ W_pq).reshape(il, H, PKD, 3),
                       rot[i0:i1]) + trans[i0:i1, None, None, :]
        qsH = qs.transpose(1, 0, 2)
        qpH = qp.reshape(il, H, PKD * 3).transpose(1, 0, 2)
        logits = (qsH @ ksT) * SCALAR_SCALE
        cross = qpH @ kpT
        bias = (pair_l.reshape(il * n, PD) @ W_pair + b_pair)
        bias = bias.reshape(il, n, H).transpose(2, 0, 1)
        logits = logits + bias * PAIR_SCALE
        q2 = np.sum(qp * qp, axis=(-1, -2))
        pdist = q2.T[:, :, None] + k2.T[:, None, :] - 2.0 * cross
        logits = logits + pdist * ((-0.5 * POINT_SCALE) * pw)[:, None, None]
        m = logits.max(axis=-1, keepdims=True)
        e = np.exp(logits - m, dtype=f)
        attn = e / e.sum(axis=-1, keepdims=True)
        rs = (attn @ vsH).transpose(1, 0, 2)
        rpt = (attn @ vpH).transpose(1, 0, 2)
        rpair = np.ascontiguousarray(attn.transpose(1, 0, 2)) @ pair_l
        rpt = rpt.reshape(il, H, PVD, 3) - trans[i0:i1, None, None, :]
        rpt_local = np.einsum('ihdc,icr->ihdr', rpt, rot[i0:i1])
        rnorm = np.sqrt(np.sum(rpt_local * rpt_local, axis=-1) + EPS)
        feats = np.concatenate([
            rs.reshape(il, H * SVD), rpt_local.reshape(il, H * PVD * 3),
            rnorm.reshape(il, H * PVD), rpair.reshape(il, H * PD)], axis=-1)
        outs.append(feats @ W_out + b_out)
    return np.concatenate(outs, axis=0)[None].astype(f)


def kernel(**inputs):
    try:
        if "nc" not in _STATE:
            import jax
            cache_dir = os.path.expanduser("~/.cache/jax_ipa_kernel")
            try:
                jax.config.update("jax_compilation_cache_dir", cache_dir)
                jax.config.update("jax_persistent_cache_min_entry_size_bytes", -1)
                jax.config.update("jax_persistent_cache_min_compile_time_secs", 0)
            except Exception:
                pass
            _STATE["nc"] = _build_program()
        from concourse.bass_utils import run_bass_kernel_spmd
        in_maps = _host_prep(inputs)
        res = run_bass_kernel_spmd(_STATE["nc"], in_maps,
                                   core_ids=list(range(N_CORES)))
        out = np.concatenate([res.results[c]["OUT"] for c in range(N_CORES)],
                             axis=0)
        return out[None].astype(np.float32)
    except Exception:
        import traceback
        traceback.print_exc()
        return _kernel_numpy(**inputs)
